# revision 28
# baseline (speedup 1.0000x reference)
"""Distributed Trainium2 kernel for nn_Attention_16947940950479.

Reference computation (B=4, S=2048, F=1024, DK=1024):
    q = x @ Wq.T + bq ; k = x @ Wk.T + bk ; v = x @ Wv.T + bv
    scores = (q @ k.T) / sqrt(DK)
    attn = softmax(scores, axis=-2)        # over the QUERY axis
    ctx = attn @ v
    out = ctx @ Wo.T + bo

Sharding (8 NeuronCores): core c = 2*b + h owns batch b, query-half h
(1024 queries). Each core computes K/V for its batch fully (duplicate
compute within the pair — cheaper than an all-gather of K/V), the Q/score/
ctx/out pipeline only for its query half. Because the softmax normalizes
over queries, scores are kept transposed [key, query] so the softmax sum
is a free-axis reduction fused into the ScalarE exp (accum_out); the only
cross-core communication is an AllReduce of the 2048 per-key denominators
within each 2-core pair ([[0,1],[2,3],[4,5],[6,7]]).

All matmuls run in bf16 with f32 PSUM accumulation. The host pre-
transposes and pre-casts x and the weights so the device does no
transposes at all: every matmul operand arrives with its contraction axis
on partitions.
"""

import os

import numpy as np
import ml_dtypes

import concourse.bass as bass
import concourse.mybir as mybir
from concourse import bacc, tile
from concourse.bass_utils import run_bass_kernel_spmd
from concourse.tile_rust import add_dep_helper

B, S, F, DK = 4, 2048, 1024, 1024
N_CORES = 8
SH = S // 2            # queries per core
NQB = SH // 512        # q blocks of 512
NKB = S // 512         # key blocks of 512 (K projection)
NKT = S // 128         # key tiles of 128
NFT = F // 128         # f tiles (contraction of projections)
NDT = DK // 128        # d tiles
SCALE = 1.0 / float(np.sqrt(DK))
BF16 = mybir.dt.bfloat16
F32 = mybir.dt.float32
BF = ml_dtypes.bfloat16

REPLICA_GROUPS = [[0, 1], [2, 3], [4, 5], [6, 7]]

_COMPILED = None
LAST_RESULTS = None


def _build():
    nc = bacc.Bacc(
        "TRN2", target_bir_lowering=False, debug=False, num_devices=N_CORES
    )
    xqT = nc.dram_tensor("xqT", [F, SH], BF16, kind="ExternalInput").ap()
    xkT = nc.dram_tensor("xkT", [F, S], BF16, kind="ExternalInput").ap()
    wqT = nc.dram_tensor("wqT", [F, DK], BF16, kind="ExternalInput").ap()
    wkN = nc.dram_tensor("wkN", [DK, F], BF16, kind="ExternalInput").ap()
    wvT = nc.dram_tensor("wvT", [F, DK], BF16, kind="ExternalInput").ap()
    woT = nc.dram_tensor("woT", [DK, F], BF16, kind="ExternalInput").ap()
    bqr = nc.dram_tensor("bqr", [128, NDT], F32, kind="ExternalInput").ap()
    bkc = nc.dram_tensor("bkc", [128, NDT], BF16, kind="ExternalInput").ap()
    bor = nc.dram_tensor("bor", [128, NFT], F32, kind="ExternalInput").ap()
    bvb = nc.dram_tensor("bvb", [128, DK], F32, kind="ExternalInput").ap()
    outT = nc.dram_tensor("outT", [F, SH], F32, kind="ExternalOutput").ap()

    with tile.TileContext(nc) as tc:
        with (
            tc.tile_pool(name="smalls", bufs=1) as smalls,
            tc.tile_pool(name="qkv", bufs=1) as qkv,
            tc.tile_pool(name="psum", bufs=8, space="PSUM") as psum,
            tc.tile_pool(name="dram", bufs=1, space="DRAM") as dram,
        ):
            bq_t = smalls.tile([128, NDT], F32, name="bq_t")
            bk_t = smalls.tile([128, NDT], BF16, name="bk_t")
            one_t = smalls.tile([1, 128], BF16, name="one_t")
            nc.vector.memset(one_t[:], 1.0)
            cq_sb = smalls.tile([1, SH], BF16, name="cq_sb")
            bo_t = smalls.tile([128, NFT], F32, name="bo_t")
            bvb_t = smalls.tile([128, DK], F32, name="bvb_t")
            dacc = smalls.tile([128, 2 * NKT], F32, name="dacc")
            den = smalls.tile([128, NKT], F32, name="den")
            deng = smalls.tile([128, NKT], F32, name="deng")
            inv = smalls.tile([128, NKT], F32, name="inv")
            nc.sync.dma_start(bq_t[:], bqr)
            nc.sync.dma_start(bk_t[:], bkc)
            nc.sync.dma_start(bo_t[:], bor)
            nc.sync.dma_start(bvb_t[:], bvb)

            qT = [qkv.tile([128, SH], BF16, name=f"qT{i}") for i in range(NDT)]
            qkT = [qkv.tile([128, SH], BF16, name=f"qkT{i}") for i in range(NFT)]
            xk_t = [qkv.tile([128, S], BF16, name=f"xk{i}") for i in range(NFT)]
            vt = [qkv.tile([128, DK], BF16, name=f"vt{i}") for i in range(NKT)]

            with tc.tile_pool(name="ph1", bufs=1) as ph1:
                xq_t = [ph1.tile([128, SH], BF16, name=f"xq{i}") for i in range(NFT)]
                wq_t = [ph1.tile([128, DK], BF16, name=f"wq{i}") for i in range(NFT)]
                wk_t = [ph1.tile([128, F], BF16, name=f"wk{i}") for i in range(NDT)]
                wv_t = [ph1.tile([128, DK], BF16, name=f"wv{i}") for i in range(NFT)]
                # DMAs in consumption order: Q operands (split in two f
                # halves to let the PE start after only 2MB has landed),
                # then K operands, then V weights.
                for half in range(2):
                    for i in range(half * NFT // 2, (half + 1) * NFT // 2):
                        r = slice(i * 128, (i + 1) * 128)
                        nc.sync.dma_start(wq_t[i][:], wqT[r, :])
                        nc.sync.dma_start(xq_t[i][:], xqT[r, :])
                for i in range(NDT):
                    r = slice(i * 128, (i + 1) * 128)
                    nc.sync.dma_start(wk_t[i][:], wkN[r, :])
                for i in range(NFT):
                    r = slice(i * 128, (i + 1) * 128)
                    nc.sync.dma_start(xk_t[i][:], xkT[r, :])
                for i in range(NFT):
                    r = slice(i * 128, (i + 1) * 128)
                    nc.sync.dma_start(wv_t[i][:], wvT[r, :])

                # Q projection: qT[d, q] = sum_f WqT[f, d] * xqT[f, q] + bq[d]
                # Chains split into f-halves (A: fi 0..3, B: fi 4..7), in
                # groups of 8 open PSUM accumulations, so the A parts only
                # need the first half of the Q operand DMAs.
                qchains = [(di, qb) for di in range(NDT) for qb in range(NQB)]
                for grp in range(0, len(qchains), 8):
                    group = qchains[grp : grp + 8]
                    qps = {}
                    for di, qb in group:
                        dsl = slice(di * 128, (di + 1) * 128)
                        qsl = slice(qb * 512, (qb + 1) * 512)
                        ps = psum.tile([128, 512], F32, name="ps", tag="ps")
                        qps[(di, qb)] = ps
                        for fi in range(NFT // 2):
                            nc.tensor.matmul(
                                ps[:], wq_t[fi][:, dsl], xq_t[fi][:, qsl],
                                start=(fi == 0), stop=False,
                            )
                    for di, qb in group:
                        dsl = slice(di * 128, (di + 1) * 128)
                        qsl = slice(qb * 512, (qb + 1) * 512)
                        ps = qps[(di, qb)]
                        for fi in range(NFT // 2, NFT):
                            nc.tensor.matmul(
                                ps[:], wq_t[fi][:, dsl], xq_t[fi][:, qsl],
                                start=False, stop=(fi == NFT - 1),
                            )
                        nc.vector.tensor_scalar_add(
                            qT[di][:, qsl], ps[:], bq_t[:, di : di + 1]
                        )

                # Associativity rewrite of the K side: instead of
                # projecting all 2048 keys (k = x@Wk.T; scores = q@k.T),
                # compute qk^T[f, q] = sum_d Wk[d, f] * q[d, q] over the
                # 1024 local queries only; scores^T = xk^T-contraction
                # with qk^T later. The q.bk rank-1 term is restored via
                # cq[q] = sum_d bk[d] q[d, q] and a K=1 matmul per score
                # chain.
                for fi in range(NFT):
                    fsl = slice(fi * 128, (fi + 1) * 128)
                    for qb in range(NQB):
                        qsl = slice(qb * 512, (qb + 1) * 512)
                        ps = psum.tile([128, 512], F32, name="ps", tag="ps")
                        for di in range(NDT):
                            nc.tensor.matmul(
                                ps[:], wk_t[di][:, fsl], qT[di][:, qsl],
                                start=(di == 0), stop=(di == NDT - 1),
                            )
                        nc.vector.tensor_copy(qkT[fi][:, qsl], ps[:])
                for qb in range(NQB):
                    qsl = slice(qb * 512, (qb + 1) * 512)
                    ps = psum.tile([1, 512], F32, name="psc", tag="ps")
                    for di in range(NDT):
                        nc.tensor.matmul(
                            ps[:], bk_t[:, di : di + 1], qT[di][:, qsl],
                            start=(di == 0), stop=(di == NDT - 1),
                        )
                    nc.vector.tensor_copy(cq_sb[0:1, qsl], ps[:])

                # V projection: v[k, d]; bias added during PSUM evacuation
                for ki in range(NKT):
                    ksl = slice(ki * 128, (ki + 1) * 128)
                    for db in range(2):
                        dsl = slice(db * 512, (db + 1) * 512)
                        ps = psum.tile([128, 512], F32, name="ps", tag="ps")
                        for fi in range(NFT):
                            nc.tensor.matmul(
                                ps[:], xk_t[fi][:, ksl], wv_t[fi][:, dsl],
                                start=(fi == 0), stop=(fi == NFT - 1),
                            )
                        nc.vector.tensor_add(vt[ki][:, dsl], ps[:], bvb_t[:, dsl])

            with tc.tile_pool(name="ph2", bufs=1) as ph2:
                p_t = [ph2.tile([128, SH], BF16, name=f"p{i}") for i in range(NKT)]
                wo_t = [ph2.tile([128, F], BF16, name=f"wo{i}") for i in range(NDT)]
                ctx_t = [ph2.tile([128, SH], BF16, name=f"ctx{i}") for i in range(NDT)]
                for i in range(NDT):
                    nc.sync.dma_start(wo_t[i][:], woT[i * 128 : (i + 1) * 128, :])

                # scores^T[k, q] -> exp(scale*.) -> p (bf16) + per-key rowsums.
                # The key axis is processed in 2 chunks of 8 k-tiles; each
                # chunk's denominator AllReduce is issued as soon as the
                # chunk's scores are done, so chunk 0's collective hides
                # under chunk 1's score matmuls and chunk 1's collective
                # hides under the ctx matmuls on chunk 0.
                NCH = 2
                CHK = NKT // NCH  # k-tiles per chunk
                prev_readback = None
                cc_ins = [
                    dram.tile([128, CHK], F32, name=f"cc_in{c}") for c in range(NCH)
                ]
                cc_outs = [
                    dram.tile([128, CHK], F32, name=f"cc_out{c}") for c in range(NCH)
                ]
                for ch in range(NCH):
                    for ki in range(ch * CHK, (ch + 1) * CHK):
                        ksl = slice(ki * 128, (ki + 1) * 128)
                        for qb in range(NQB):
                            qsl = slice(qb * 512, (qb + 1) * 512)
                            ps = psum.tile([128, 512], F32, name="ps", tag="ps")
                            for fi in range(NFT):
                                nc.tensor.matmul(
                                    ps[:], xk_t[fi][:, ksl], qkT[fi][:, qsl],
                                    start=(fi == 0), stop=False,
                                )
                            nc.tensor.matmul(
                                ps[:], one_t[0:1, :], cq_sb[0:1, qsl],
                                start=False, stop=True,
                            )
                            j = qb * NKT + ki
                            nc.scalar.activation(
                                p_t[ki][:, qsl], ps[:],
                                mybir.ActivationFunctionType.Exp,
                                scale=SCALE,
                                accum_out=dacc[:, j : j + 1],
                            )
                    # local chunk denominators -> pair AllReduce -> 1/x
                    csl = slice(ch * CHK, (ch + 1) * CHK)
                    nc.vector.tensor_add(
                        den[:, csl],
                        dacc[:, ch * CHK : (ch + 1) * CHK],
                        dacc[:, NKT + ch * CHK : NKT + (ch + 1) * CHK],
                    )
                    if ch > 0:
                        # Dummy AllReduce between the real ones: keeps the
                        # collective firmware (TOPSP ncfw) warm so chunk
                        # ch's AllReduce starts in ~1us instead of ~11us.
                        warm = nc.gpsimd.collective_compute(
                            "AllReduce",
                            mybir.AluOpType.add,
                            replica_groups=REPLICA_GROUPS,
                            ins=[cc_ins[ch - 1].opt()],
                            outs=[cc_outs[ch - 1].opt()],
                        )
                    cin_dma = nc.gpsimd.dma_start(cc_ins[ch][:], den[:, csl])
                    if ch > 0 and prev_readback is not None:
                        # Keep the gpsimd stream in dataflow order: chunk
                        # ch's bounce write must not be scheduled ahead of
                        # chunk ch-1's result readback, else the readback
                        # (and the dependent attn scaling) stalls behind
                        # chunk ch's exp tail.
                        add_dep_helper(
                            cin_dma.ins, prev_readback.ins, False,
                            "AR bounce order: readback before next chunk in",
                        )
                    cc = nc.gpsimd.collective_compute(
                        "AllReduce",
                        mybir.AluOpType.add,
                        replica_groups=REPLICA_GROUPS,
                        ins=[cc_ins[ch].opt()],
                        outs=[cc_outs[ch].opt()],
                    )
                    if ch > 0:
                        add_dep_helper(
                            cc.ins, warm.ins, False,
                            "collective order: warmup AR before chunk AR",
                        )
                    prev_readback = nc.gpsimd.dma_start(deng[:, csl], cc_outs[ch][:])
                    nc.vector.reciprocal(inv[:, csl], deng[:, csl])
                    # attn^T = p * inv[k]  (per-partition scalar, in place)
                    for ki in range(ch * CHK, (ch + 1) * CHK):
                        nc.vector.tensor_scalar_mul(
                            p_t[ki][:], p_t[ki][:], inv[:, ki : ki + 1]
                        )

                # ctx^T[d, q] = sum_k v[k, d] * attn^T[k, q]
                # Chains split by k-chunk (A: ki 0..7, B: ki 8..15) in groups
                # of 8 open PSUM accumulations: the A parts only need chunk-0
                # attention weights, so they execute while chunk 1's
                # denominator AllReduce is still in flight.
                cchains = [(di, qb) for di in range(NDT) for qb in range(NQB)]
                for grp in range(0, len(cchains), 8):
                    group = cchains[grp : grp + 8]
                    cps = {}
                    for di, qb in group:
                        dsl = slice(di * 128, (di + 1) * 128)
                        qsl = slice(qb * 512, (qb + 1) * 512)
                        ps = psum.tile([128, 512], F32, name="ps", tag="ps")
                        cps[(di, qb)] = ps
                        for ki in range(CHK):
                            nc.tensor.matmul(
                                ps[:], vt[ki][:, dsl], p_t[ki][:, qsl],
                                start=(ki == 0), stop=False,
                            )
                    for di, qb in group:
                        dsl = slice(di * 128, (di + 1) * 128)
                        qsl = slice(qb * 512, (qb + 1) * 512)
                        ps = cps[(di, qb)]
                        for ki in range(CHK, NKT):
                            nc.tensor.matmul(
                                ps[:], vt[ki][:, dsl], p_t[ki][:, qsl],
                                start=False, stop=(ki == NKT - 1),
                            )
                        nc.vector.tensor_copy(ctx_t[di][:, qsl], ps[:])

                # out^T[f', q] = sum_d WoT[d, f'] * ctx^T[d, q] + bo[f']
                for fi in range(NFT):
                    fsl = slice(fi * 128, (fi + 1) * 128)
                    for qb in range(NQB):
                        qsl = slice(qb * 512, (qb + 1) * 512)
                        ps = psum.tile([128, 512], F32, name="ps", tag="ps")
                        for di in range(NDT):
                            nc.tensor.matmul(
                                ps[:], wo_t[di][:, fsl], ctx_t[di][:, qsl],
                                start=(di == 0), stop=(di == NDT - 1),
                            )
                        ot = ph2.tile([128, 512], F32, name="ost", tag="ost", bufs=3)
                        nc.vector.tensor_scalar_add(ot[:], ps[:], bo_t[:, fi : fi + 1])
                        nc.sync.dma_start(outT[fsl, qsl], ot[:])

    nc.compile()
    return nc


def _get_compiled():
    global _COMPILED
    if _COMPILED is None:
        _COMPILED = _build()
    return _COMPILED


def kernel(x, Wq, bq, Wk, bk, Wv, bv, Wo, bo):
    global LAST_RESULTS
    nc = _get_compiled()

    x = np.asarray(x, dtype=np.float32)
    wqT = np.ascontiguousarray(np.asarray(Wq, np.float32).T).astype(BF)
    wkN = np.ascontiguousarray(np.asarray(Wk, np.float32)).astype(BF)
    wvT = np.ascontiguousarray(np.asarray(Wv, np.float32).T).astype(BF)
    woT = np.ascontiguousarray(np.asarray(Wo, np.float32).T).astype(BF)
    bqr = np.ascontiguousarray(np.asarray(bq, np.float32).reshape(NDT, 128).T)
    bkc = np.ascontiguousarray(np.asarray(bk, np.float32).reshape(NDT, 128).T).astype(BF)
    bor = np.ascontiguousarray(np.asarray(bo, np.float32).reshape(NFT, 128).T)
    bvb = np.ascontiguousarray(
        np.broadcast_to(np.asarray(bv, np.float32)[None, :], (128, DK))
    )

    shared = {
        "wqT": wqT, "wkN": wkN, "wvT": wvT, "woT": woT,
        "bqr": bqr, "bkc": bkc, "bor": bor, "bvb": bvb,
    }
    xkT_b = [np.ascontiguousarray(x[b].T).astype(BF) for b in range(B)]
    in_maps = []
    for c in range(N_CORES):
        b, h = c // 2, c % 2
        xqT_c = np.ascontiguousarray(x[b, h * SH : (h + 1) * SH, :].T).astype(BF)
        in_maps.append({"xqT": xqT_c, "xkT": xkT_b[b], **shared})

    res = run_bass_kernel_spmd(nc, in_maps, list(range(N_CORES)))
    LAST_RESULTS = res

    out = np.empty((B, S, F), np.float32)
    for c in range(N_CORES):
        b, h = c // 2, c % 2
        out[b, h * SH : (h + 1) * SH, :] = res.results[c]["outT"].T
    return out
